# revision 67
# baseline (speedup 1.0000x reference)
"""FAVOR+ (Performer) multi-head causal attention — Trainium2 Bass kernel v3.

Sharding: 8 cores = 4 batches x 2 head-groups (4 heads each); each core
computes its head-group's attention and a row-parallel partial of w_o;
the host sums the two partials per batch (the "all-gather" of the hint)
and adds the folded output constant.

Math vs the reference (all exact up to bf16 rounding):
  * qp softmax-kernel stabilizer/eps and per-query diag_q cancel in
    num/den -> qp = exp(dd_q) raw.
  * kp's global stabilizer cancels; the per-KEY diag factor e[l] =
    exp(-0.5|k~|^2) is linear in each key's contribution, so it is folded
    into the e-scaled augmented v rows (va_s = [v|1|1]*e[l]):
      - state sums:  kplc_raw^T va_s  ==  kplc_true^T va
      - in-chunk:    va_s^T (psA_raw . mask)  ==  va^T (A_true . mask)
    This removes diag handling from ALL feature tensors, enabling
    bias-free 1024-wide merged exps.
  * bv shifts attention output by exactly bv (attention weights sum to
    1), so it is dropped on-chip and folded into the host-side constant
    (Wo@bv + bo).

Performance design (driven by the TimelineSim cost model):
  * matmul cost = out-columns only (contract dim free); PE SEQ dispatch
    ~96ns/matmul -> fewer, wider ops; no PE broadcast/transpose tricks.
  * kp[m,l] is the elementwise transpose of kplc[l,m] (the same exp'd
    values): produced by the DMA-xbar transpose (SBUF->SBUF) instead of
    a second matmul+exp pass. Out AP must be 3D with 32B-aligned block
    strides (144 bf16 elems/chunk block) or the xbar skews (verified on
    hardware); saves ~16us Act + ~7us PE per core.
  * PSUM accumulation groups are per-BANK: every 512-f32 bank gets its
    own start..stop bracket.
  * prep state sums accumulate IN PSUM across chunks (start only on the
    first chunk); Act snapshots psum->SBUF, removing the DVE prefix-add
    chain.
  * den reciprocal is broadcast across partitions on the (otherwise
    idle) Pool engine via partition_broadcast; att = num*rcpB on DVE.
  * x loads split into 1024-col halves and the weight blob into 3
    stages so the serial DMA device feeds projections ASAP; per-head
    kp transposes ride the DMA device between load stages.
  * schedule: phase A (DMA-shadowed) does projections + gen(0,1)@1024;
    the spine is four solo scans, each carrying the next head's
    generation + fillers (v units, e-scales, preps, w_o) paced per
    chunk; masks-first within each scan decouples psA/mask progress
    from the later-gated den chains.
"""
import numpy as np

B, L, DIM, H, DK, M = 4, 2048, 512, 8, 64, 256
HPC = 4            # heads per core
CW = 256           # scan chunk width (queries per chunk)
NCC = L // CW      # 8
NC2 = L // 128     # 16
LT = 512
NLT = L // LT

# constant-blob column offsets (bf16 columns)
_OFF_WQ = 0
_OFF_WK = 1024
_OFF_BQK = 2048
_B1 = 2052          # end of blob part 1
_OFF_PROJT = 2052
_OFF_BD = 2308
_OFF_WV = 2316
_OFF_MSK = 3340
_OFF_WO2 = 3724
_CB = 4748

_COMPILED = None
_DEBUG = False


def _build():
    import concourse.bacc as bacc
    import concourse.mybir as mybir
    from concourse.tile import TileContext

    f32 = mybir.dt.float32
    bf16 = mybir.dt.bfloat16
    EXP = mybir.ActivationFunctionType.Exp

    nc = bacc.Bacc("TRN2", target_bir_lowering=False, debug=False,
                   enable_asserts=False, num_devices=8)

    def din(name, shape, dt=bf16):
        return nc.dram_tensor(name, shape, dt, kind="ExternalInput").ap()

    cblob = din("cblob", [128, _CB])
    xq = din("xq", [512, L])
    xk = din("xk", [512, L])
    xv = din("xv", [512, L])
    outT = nc.dram_tensor("outT", [512, L], bf16, kind="ExternalOutput").ap()
    dbg = {}
    if _DEBUG:
        for nm, shp in (("qp0", [128, L]), ("kp0", [128, L]),
                        ("kplc0", [128, NC2 * 256]), ("vas", [128, NC2 * 264]),
                        ("e", [128, 64]), ("att0", [128, L]),
                        ("att1", [128, L]), ("S0", [128, 924])):
            dt = f32 if nm == "e" else bf16
            dbg[nm] = nc.dram_tensor("dbg_" + nm, shp, dt,
                                     kind="ExternalOutput").ap()

    with TileContext(nc) as tc, nc.allow_low_precision(
            reason="bf16 scan pipeline, validated ~5e-3 rel err vs f32 "
                   "reference (tolerance 2e-2)"):
        with (
            tc.tile_pool(name="const", bufs=1) as cpool,
            tc.tile_pool(name="persist", bufs=1) as ppool,
        ):
            psS_box = [None]
            # ---- constants ----
            blob = cpool.tile([128, _CB], bf16, tag="cblob")
            nc.sync.dma_start(blob[:, 0:_B1], cblob[:, 0:_B1])

            def bs(off, w):
                return blob[:, off:off + w]

            c_wq = [bs(_OFF_WQ + i * 256, 256) for i in range(4)]
            c_wk = [bs(_OFF_WK + i * 256, 256) for i in range(4)]
            c_bqk = cpool.tile([128, 4], f32, tag="bqk")
            c_bq = c_bqk[:, 0:2]
            c_bk = c_bqk[:, 2:4]
            c_projT = bs(_OFF_PROJT, 256)
            c_bd = bs(_OFF_BD, 8)
            c_wv = [bs(_OFF_WV + i * 256, 256) for i in range(4)]
            c_msk = bs(_OFF_MSK, 384)
            c_wo2 = [bs(_OFF_WO2 + j * 512, 512) for j in range(2)]

            # persistent activations
            t_qT = [ppool.tile([128, L], bf16, name=f"qT{i}", tag=f"qT{i}")
                    for i in range(2)]
            t_kT = [ppool.tile([128, L], bf16, name=f"kT{i}", tag=f"kT{i}")
                    for i in range(2)]
            t_sq = [ppool.tile([128, L], bf16, name=f"sq{i}", tag=f"sq{i}")
                    for i in range(2)]
            t_v = ppool.tile([128, NC2 * 264], bf16, tag="vall")
            t_vs = ppool.tile([128, NC2 * 264], bf16, tag="vscl")
            t_e = ppool.tile([128, 64], f32, tag="eksc")

            # ---- x loads (1024-col halves so projections start sooner;
            # blob2 split so each piece lands just before its consumers) ----
            t_x = {}
            for nm, src in (("q", xq), ("k", xk), ("v", xv)):
                for half in range(2):
                    hs = slice(half * 1024, (half + 1) * 1024)
                    for i in range(4):
                        if half == 0:
                            xt = ppool.tile([128, L], bf16, name=f"x{nm}{i}",
                                            tag=f"x{nm}{i}")
                            t_x[(nm, i)] = xt
                        nc.sync.dma_start(
                            t_x[(nm, i)][:, hs],
                            src[128 * i:128 * (i + 1), hs])
                if nm == "q":
                    nc.sync.dma_start(blob[:, _OFF_PROJT:_OFF_WV],
                                      cblob[:, _OFF_PROJT:_OFF_WV])
                elif nm == "k":
                    nc.sync.dma_start(blob[:, _OFF_WV:_OFF_MSK],
                                      cblob[:, _OFF_WV:_OFF_MSK])
            nc.sync.dma_start(blob[:, _OFF_MSK:_CB], cblob[:, _OFF_MSK:_CB])
            nc.vector.tensor_copy(c_bqk[:, :], bs(_OFF_BQK, 4))

            v_r4 = t_v[:, :].rearrange("p (c h x) -> p c h x", h=4, x=66)
            vs_r4 = t_vs[:, :].rearrange("p (c h x) -> p c h x", h=4, x=66)
            nc.vector.memset(v_r4[:, :, :, 64:66], 1.0)

            # ---- wide-psum pool for the pre-scan phase ----
            psG_c = tc.tile_pool(name="psG", bufs=2, space="PSUM")
            psG_p = psG_c.__enter__()

            def psG():
                return psG_p.tile([128, 1024], f32, name="psG", tag="psG")

            def proj_unit(nm, half, ltp, wide=True):
                wgt, dst, bias = ((c_wq, t_qT, c_bq) if nm == "q"
                                  else (c_wk, t_kT, c_bk))
                n = 2 if wide else 1
                ps = psG() if wide else psG2()
                for j in range(n):
                    lt = n * ltp + j
                    ls = slice(lt * LT, (lt + 1) * LT)
                    for kt in range(4):
                        nc.tensor.matmul(
                            ps[:, j * 512:(j + 1) * 512],
                            wgt[kt][:, 128 * half:128 * (half + 1)],
                            t_x[(nm, kt)][:, ls],
                            start=(kt == 0), stop=(kt == 3))
                ws = slice(ltp * 512 * n, (ltp + 1) * 512 * n)
                nc.vector.tensor_scalar_add(dst[half][:, ws],
                                            ps[:, 0:512 * n],
                                            bias[:, half:half + 1])
                if nm == "k":
                    nc.vector.tensor_mul(t_sq[half][:, ws],
                                         dst[half][:, ws],
                                         dst[half][:, ws])

            def ksc_unit(u, hp, wide=True):
                # chunks 8u..8u+7, head-pair hp: one bracket, 8 matmuls into
                # cols 2j:2j+2, one exp (strided) into t_e = diag factors
                ps = psG() if wide else psG2()
                for j in range(8):
                    ch = 8 * u + j
                    cs = slice(ch * 128, (ch + 1) * 128)
                    nc.tensor.matmul(
                        ps[:, 2 * j:2 * j + 2], t_sq[hp][:, cs],
                        c_bd[:, 4 * hp + 2 * hp:4 * hp + 2 * hp + 2],
                        start=(j == 0), stop=(j == 7))
                e_out = t_e[:, 32 * u:32 * (u + 1)].rearrange(
                    "p (c h) -> p c h", h=4)[:, :, 2 * hp:2 * hp + 2]
                ps_r = ps[:, 0:16].rearrange("p (c h) -> p c h", h=2)
                nc.scalar.activation(e_out, ps_r, EXP)

            def v_unit(u):
                # chunks 4u..4u+3 -> one [128,1024] psum -> one strided copy
                ps = psG()
                for j in range(4):
                    ch = 4 * u + j
                    cs = slice(ch * 128, (ch + 1) * 128)
                    for kt in range(4):
                        nc.tensor.matmul(
                            ps[:, j * 256:(j + 1) * 256],
                            t_x[("v", kt)][:, cs], c_wv[kt][:, :],
                            start=(j == 0 and kt == 0),
                            stop=(j == 3 and kt == 3))
                ps_r = ps[:, :].rearrange("p (c h x) -> p c h x", h=4, x=64)
                nc.vector.tensor_copy(v_r4[:, 4 * u:4 * (u + 1), :, 0:64],
                                      ps_r[:, :, :, :])

            def vs_unit(ch, h):
                # fold the per-key diag factor into the augmented v rows
                # (heads 0/1 on Act via Copy-with-scale; 2/3 on Pool)
                if h < 2:
                    nc.scalar.activation(
                        vs_r4[:, ch, h, 0:66], v_r4[:, ch, h, 0:66],
                        mybir.ActivationFunctionType.Copy,
                        scale=t_e[:, 4 * ch + h:4 * ch + h + 1])
                else:
                    nc.gpsimd.tensor_scalar_mul(
                        vs_r4[:, ch, h, 0:66], v_r4[:, ch, h, 0:66],
                        t_e[:, 4 * ch + h:4 * ch + h + 1])

            # ---- per-head tile pools ----
            hctx = (tc.tile_pool(name="headbuf", bufs=3),
                    tc.tile_pool(name="work", bufs=3),
                    tc.tile_pool(name="attn", bufs=1))
            hpool, wpool, apool = [c.__enter__() for c in hctx]
            octx = tc.tile_pool(name="outp", bufs=6)
            opool = octx.__enter__()

            t_att = [apool.tile([128, L], bf16, name=f"att{i}", tag=f"att{i}")
                     for i in range(2)]

            heads = {}

            def gen_units(h, wide, kp_direct=False):
                hh = h // 2
                hr = slice(64 * (h % 2), 64 * (h % 2) + 64)
                t_qp = [hpool.tile([128, L], bf16, name=f"qp{i}", tag=f"qp{i}")
                        for i in range(2)]
                # kp[mh]: for wide heads, direct [m, l] in cols 0:2048; for
                # narrow heads, DMA-transposed kplc in 144-element chunk
                # blocks (32B-aligned strides; xbar skews misaligned ones).
                t_kp = [hpool.tile([128, NC2 * 144], bf16, name=f"kp{i}",
                                   tag=f"kp{i}") for i in range(2)]
                t_kplc = hpool.tile([128, NC2 * 256], bf16, tag="kplc",
                                    bufs=2)
                t_S = hpool.tile([128, (NCC - 1) * 132], bf16, tag="S")

                if kp_direct:
                    def kpsl(mh, c128):
                        return t_kp[mh][:, c128 * 128:(c128 + 1) * 128]
                else:
                    def kpsl(mh, c128):
                        return t_kp[mh][:, c128 * 144:c128 * 144 + 128]
                heads[h] = (t_qp, kpsl, t_kplc, t_S)
                kplc_r = t_kplc[:, :].rearrange("p (h c m) -> p c h m",
                                                h=2, c=NC2, m=128)

                def qk_unit(sr, dst, mh, ltp):
                    ms = slice(128 * mh, 128 * (mh + 1))
                    ps = psG() if wide else psG2()
                    n = 2 if wide else 1
                    for j in range(n):
                        lt = n * ltp + j
                        ls = slice(lt * LT, (lt + 1) * LT)
                        nc.tensor.matmul(ps[:, j * 512:(j + 1) * 512],
                                         c_projT[hr, ms], sr[hh][hr, ls],
                                         start=True, stop=True)
                    ws = slice(ltp * 512 * n, (ltp + 1) * 512 * n)
                    nc.scalar.activation(dst[mh][:, ws],
                                         ps[:, 0:512 * n], EXP)

                def kplc_unit(u):
                    ps = psG() if wide else psG2()
                    n = 4 if wide else 2
                    for j in range(n):
                        ch = n * u + j
                        cs = slice(ch * 128, (ch + 1) * 128)
                        nc.tensor.matmul(ps[:, j * 256:(j + 1) * 256],
                                         t_kT[hh][hr, cs], c_projT[hr, :],
                                         start=(j % 2 == 0),
                                         stop=(j % 2 == 1))
                    ps_r = ps[:, 0:256 * n].rearrange(
                        "p (c h m) -> p c h m", h=2, m=128)
                    nc.scalar.activation(
                        kplc_r[:, n * u:n * (u + 1), :, :], ps_r, EXP)

                def kp_transpose():
                    # kp = elementwise transpose of kplc via the DMA xbar:
                    # out[m, ch, l] = kplc[l, mh*2048 + ch*128 + m]
                    for mh in range(2):
                        kp3 = t_kp[mh][:, :].rearrange(
                            "p (c l) -> p c l", c=NC2, l=144)[:, :, 0:128]
                        nc.scalar.dma_start(
                            kp3, t_kplc[:, mh * 2048:(mh + 1) * 2048],
                            transpose=True)

                nl = 2 if wide else 4
                nk = 4 if wide else 8
                qps = [lambda mh=mh, ltp=ltp: qk_unit(t_qT, t_qp, mh, ltp)
                       for mh in range(2) for ltp in range(nl)]
                kps = [lambda mh=mh, ltp=ltp: qk_unit(t_kT, t_kp, mh, ltp)
                       for mh in range(2) for ltp in range(nl)]
                kplcs = [lambda u=u: kplc_unit(u) for u in range(nk)]
                return qps, kps, kplcs, kp_transpose

            def va(h, c128):
                o = c128 * 264 + h * 66
                return t_vs[:, o:o + 66]

            def prep_chunk(h, store, cc):
                t_qp, kpsl, t_kplc, t_S = heads[h]
                if cc < NCC - 1:
                    c0, c1 = 2 * cc, 2 * cc + 1
                    psS = psS_box[0].tile([128, 132], f32, name="psS", tag="psS")
                    for mh in range(2):
                        r = slice(66 * mh, 66 * mh + 66)
                        nc.tensor.matmul(
                            psS[:, r],
                            t_kplc[:, mh * 2048 + c0 * 128:
                                   mh * 2048 + c0 * 128 + 128],
                            va(h, c0), start=(mh == 0), stop=False)
                        nc.tensor.matmul(
                            psS[:, r],
                            t_kplc[:, mh * 2048 + c1 * 128:
                                   mh * 2048 + c1 * 128 + 128],
                            va(h, c1), start=False, stop=(mh == 1))
                    store[(h, cc, 'psS')] = psS
                if cc == 0:
                    return
                dst = t_S[:, (cc - 1) * 132:cc * 132]
                prev = store.pop((h, cc - 1, 'psS'))
                if cc == 1:
                    nc.vector.tensor_copy(dst, prev[:, :])
                else:
                    nc.vector.tensor_add(
                        dst, t_S[:, (cc - 2) * 132:(cc - 1) * 132],
                        prev[:, :])

            def preps(h):
                st = {}
                return [lambda cc=cc, st=st: prep_chunk(h, st, cc)
                        for cc in range(NCC)]

            def emit_psA(h, store, cc):
                t_qp, kpsl, _, _ = heads[h]
                qs = slice(cc * CW, (cc + 1) * CW)
                qhi = slice(cc * CW + 128, (cc + 1) * CW)
                psA = psA_p.tile([128, 384], f32, name="psA", tag="psA")
                nc.tensor.matmul(psA[:, 0:256], kpsl(0, 2 * cc),
                                 t_qp[0][:, qs], start=True, stop=False)
                nc.tensor.matmul(psA[:, 0:256], kpsl(1, 2 * cc),
                                 t_qp[1][:, qs], start=False, stop=False)
                nc.tensor.matmul(psA[:, 256:384], kpsl(0, 2 * cc + 1),
                                 t_qp[0][:, qhi], start=False, stop=False)
                nc.tensor.matmul(psA[:, 256:384], kpsl(1, 2 * cc + 1),
                                 t_qp[1][:, qhi], start=False, stop=True)
                store[(h, cc, 'psA')] = psA

            def emit_mask(h, store, cc):
                atm = wpool.tile([128, 384], bf16, name="atm", tag="atm",
                                 bufs=8)
                psA = store.pop((h, cc, 'psA'))
                nc.vector.tensor_mul(atm[:, :], psA[:, :], c_msk[:, :])
                store[(h, cc, 'atm')] = atm

            def emit_nd(h, store, cc):
                t_qp, kpsl, t_kplc, t_S = heads[h]
                att = t_att[h // 2]
                arow = slice(64 * (h % 2), 64 * (h % 2) + 64)
                qs = slice(cc * CW, (cc + 1) * CW)
                c0, c1 = 2 * cc, 2 * cc + 1
                atm = store.pop((h, cc, 'atm'))
                nd = psND_p.tile([128, 256], f32, name="nd", tag="psND")
                if cc > 0:
                    S_src = t_S[:, (cc - 1) * 132:cc * 132]
                    nc.tensor.matmul(nd[0:66, 0:256], S_src[:, 0:66],
                                     t_qp[0][:, qs], start=True, stop=False)
                    nc.tensor.matmul(nd[0:66, 0:256], S_src[:, 66:132],
                                     t_qp[1][:, qs], start=False, stop=False)
                    nc.tensor.matmul(nd[0:66, 128:256], va(h, c1),
                                     atm[:, 256:384], start=False, stop=False)
                    nc.tensor.matmul(nd[0:66, 0:256], va(h, c0),
                                     atm[:, 0:256], start=False, stop=True)
                else:
                    nc.tensor.matmul(nd[0:66, 0:256], va(h, c0),
                                     atm[:, 0:256], start=True, stop=False)
                    nc.tensor.matmul(nd[0:66, 128:256], va(h, c1),
                                     atm[:, 256:384], start=False, stop=True)
                t_rcp = wpool.tile([1, 256], bf16, name="t_rcp", tag="rcp",
                                   bufs=8)
                nc.vector.reciprocal(t_rcp[:, :], nd[64:65, 0:256])
                rcpB = wpool.tile([64, 256], bf16, name="rcpB", tag="rcpB",
                                  bufs=8)
                nc.gpsimd.partition_broadcast(rcpB[:, :], t_rcp[:, :])
                nc.vector.tensor_mul(att[arow, qs], nd[0:64, 0:256],
                                     rcpB[:, :])

            def emit_nd2(h, store, pp):
                # two chunks (2*pp, 2*pp+1) in one [128,512] psum bank:
                # halves the recip/bcast/att op counts and deepens the den
                # pipeline to 4 chunks with 2 pool bufs
                t_qp, kpsl, t_kplc, t_S = heads[h]
                att = t_att[h // 2]
                arow = slice(64 * (h % 2), 64 * (h % 2) + 64)
                nd = psND_p.tile([128, 512], f32, name="nd", tag="psND")
                for j in range(2):
                    cc = 2 * pp + j
                    qs = slice(cc * CW, (cc + 1) * CW)
                    c0, c1 = 2 * cc, 2 * cc + 1
                    o = 256 * j
                    atm = store.pop((h, cc, 'atm'))
                    if cc > 0:
                        S_src = t_S[:, (cc - 1) * 132:cc * 132]
                        nc.tensor.matmul(nd[0:66, o:o + 256], S_src[:, 0:66],
                                         t_qp[0][:, qs], start=True,
                                         stop=False)
                        nc.tensor.matmul(nd[0:66, o:o + 256],
                                         S_src[:, 66:132], t_qp[1][:, qs],
                                         start=False, stop=False)
                        nc.tensor.matmul(nd[0:66, o + 128:o + 256], va(h, c1),
                                         atm[:, 256:384], start=False,
                                         stop=False)
                        nc.tensor.matmul(nd[0:66, o:o + 256], va(h, c0),
                                         atm[:, 0:256], start=False,
                                         stop=True)
                    else:
                        nc.tensor.matmul(nd[0:66, o:o + 256], va(h, c0),
                                         atm[:, 0:256], start=True,
                                         stop=False)
                        nc.tensor.matmul(nd[0:66, o + 128:o + 256], va(h, c1),
                                         atm[:, 256:384], start=False,
                                         stop=True)
                t_rcp = wpool.tile([1, 512], bf16, name="t_rcp", tag="rcp",
                                   bufs=4)
                nc.vector.reciprocal(t_rcp[:, :], nd[64:65, 0:512])
                rcpB = wpool.tile([64, 512], bf16, name="rcpB", tag="rcpB",
                                  bufs=4)
                nc.gpsimd.partition_broadcast(rcpB[:, :], t_rcp[:, :])
                qs2 = slice(2 * pp * CW, (2 * pp + 2) * CW)
                nc.vector.tensor_mul(att[arow, qs2], nd[0:64, 0:512],
                                     rcpB[:, :])

            def scan(h, units=None, upc=3):
                # masks-first: all psA+mask pairs run before any nd, so the
                # (late-gated) den chains never block psA progress on PE/DVE
                store = {}
                emit_psA(h, store, 0)
                for cc in range(NCC):
                    if cc + 1 < NCC:
                        emit_psA(h, store, cc + 1)
                    emit_mask(h, store, cc)
                for cc in range(NCC):
                    emit_nd(h, store, cc)
                    if units:
                        for u in units[cc * upc:(cc + 1) * upc]:
                            u()

            def emit_wo(lt, ops=(0, 1)):
                # per-osub psums/copies, but osub pairs share one t_o tile
                # and one 3D out-DMA (halves DMA issue + sem overheads)
                ls = slice(lt * LT, (lt + 1) * LT)
                for op in ops:
                    t_o = opool.tile([128, 1024], bf16, name="t_o",
                                     tag="outT")
                    for j in range(2):
                        osub = 2 * op + j
                        os_ = slice(128 * osub, 128 * (osub + 1))
                        ps = psG2_p.tile([128, 512], f32, name="ps",
                                         tag="psG2")
                        nc.tensor.matmul(ps[:, :], c_wo2[0][:, os_],
                                         t_att[0][:, ls],
                                         start=True, stop=False)
                        nc.tensor.matmul(ps[:, :], c_wo2[1][:, os_],
                                         t_att[1][:, ls],
                                         start=False, stop=True)
                        nc.scalar.copy(t_o[:, j * 512:(j + 1) * 512],
                                       ps[:, :])
                    o3 = outT[:, ls].rearrange(
                        "(o p) l -> p o l", p=128)[:, 2 * op:2 * op + 2, :]
                    i3 = t_o[:, :].rearrange("p (o l) -> p o l", o=2)
                    nc.sync.dma_start(o3, i3)

            # ================= emission schedule =================
            # Phase A (DMA-shadowed): ALL projections + gen(0) + gen(1)
            # (direct kp via matmul+exp, 1024-wide) + ksc. The x loads take
            # ~20us on the serial DMA device; this compute hides under them.
            proj_unit("q", 0, 0)
            proj_unit("q", 0, 1)
            g0qps, g0kps, g0kplcs, g0tp = gen_units(0, wide=True)
            g0qps[0]()
            g0qps[2]()
            proj_unit("k", 0, 0)
            g0qps[1]()
            proj_unit("k", 0, 1)
            g0qps[3]()
            ksc_unit(0, 0)
            ksc_unit(1, 0)
            g0kplcs[0]()
            g0kplcs[1]()
            g0kplcs[2]()
            g0kplcs[3]()
            g0tp()
            g1qps, g1kps, g1kplcs, g1tp = gen_units(1, wide=True)
            proj_unit("q", 1, 0)
            g1kplcs[0]()
            g1kplcs[1]()
            proj_unit("q", 1, 1)
            g1kplcs[2]()
            g1kplcs[3]()
            g1tp()
            proj_unit("k", 1, 0)
            g1qps[0]()
            g1qps[2]()
            proj_unit("k", 1, 1)
            g1qps[1]()
            g1qps[3]()
            ksc_unit(0, 1)
            ksc_unit(1, 1)
            if _DEBUG:
                t_qp0, kpsl0, t_kplc0, _ = heads[0]
                nc.sync.dma_start(dbg["qp0"][0:128, :], t_qp0[0][:, :])
                nc.sync.dma_start(dbg["kplc0"], t_kplc0[:, :])
                nc.sync.dma_start(dbg["e"], t_e[:, :])
                kp00 = heads[0][1](0, 0)
                for c in range(NC2):
                    nc.sync.dma_start(dbg["kp0"][:, c * 128:(c + 1) * 128],
                                      heads[0][1](0, c))

            # Spine: four solo scans, chunk-paced fillers.
            psG_c.__exit__(None, None, None)
            psA_c = tc.tile_pool(name="psA", bufs=2, space="PSUM")
            psA_p = psA_c.__enter__()
            psND_c = tc.tile_pool(name="psND", bufs=3, space="PSUM")
            psND_p = psND_c.__enter__()
            psND_p._nd_tag = "psND"
            psG2_c = tc.tile_pool(name="psG2", bufs=2, space="PSUM")
            psG2_p = psG2_c.__enter__()
            psSb_c = tc.tile_pool(name="psSb", bufs=1, space="PSUM")
            psS_box[0] = psSb_c.__enter__()

            def psG2():
                return psG2_p.tile([128, 512], f32, name="ps", tag="psG2")

            def v2_unit(j):
                # 2-chunk v unit @psG2, Act evacuation (Act idle in sc0)
                ps = psG2()
                for i in range(2):
                    ch = 2 * j + i
                    cs = slice(ch * 128, (ch + 1) * 128)
                    for kt in range(4):
                        nc.tensor.matmul(
                            ps[:, i * 256:(i + 1) * 256],
                            t_x[("v", kt)][:, cs], c_wv[kt][:, :],
                            start=(i == 0 and kt == 0),
                            stop=(i == 1 and kt == 3))
                ps_r = ps[:, :].rearrange("p (c h x) -> p c h x", h=4, x=64)
                nc.scalar.copy(v_r4[:, 2 * j:2 * (j + 1), :, 0:64],
                               ps_r[:, :, :, :])

            def nop():
                pass

            def vsg(j, hp):
                # e-scales for v-unit j's chunks, head pair hp
                def f():
                    for hh2 in (2 * hp, 2 * hp + 1):
                        vs_unit(2 * j, hh2)
                        vs_unit(2 * j + 1, hh2)
                return f

            def v2_unit_l(j):
                return lambda: v2_unit(j)

            def vsg2(j):
                return vsg(j, 1)

            def wo_unit(lt, ops=(0, 1)):
                return lambda: emit_wo(lt, ops)

            # sc0: v + head-0/1 e-scales + preps(0)
            p0 = preps(0)
            sc0 = [v2_unit_l(0), v2_unit_l(1), vsg(0, 0), vsg(1, 0)]
            for j in range(2, 8):
                sc0 += [v2_unit_l(j), vsg(j, 0)]
            for u in sc0:
                u()
            for cc in range(NCC):
                p0[cc]()
            scan(0, units=None)

            # sc1: preps(1) + gen(2) + its transpose + head-2/3 e-scales
            p1 = preps(1)
            g2qps, g2kps, g2kplcs, g2tp = gen_units(2, wide=False)
            sc1 = [p1[0], p1[1], g2kplcs[0], g2qps[0],
                   p1[2], g2kplcs[1], g2qps[1], vsg2(0),
                   p1[3], g2kplcs[2], g2qps[2], vsg2(1),
                   p1[4], g2kplcs[3], g2qps[3], vsg2(2),
                   p1[5], g2kplcs[4], g2qps[4], vsg2(3),
                   p1[6], g2kplcs[5], g2qps[5], vsg2(4),
                   p1[7], g2kplcs[6], g2qps[6], vsg2(5),
                   g2kplcs[7], g2tp, g2qps[7], vsg2(6), vsg2(7)]
            scan(1, units=sc1, upc=6)

            # sc2: preps(2) + gen(3) + its transpose
            p2 = preps(2)
            g3qps, g3kps, g3kplcs, g3tp = gen_units(3, wide=False)
            sc2 = [p2[0], p2[1], g3kplcs[0], g3qps[0],
                   p2[2], g3kplcs[1], g3qps[1],
                   p2[3], g3kplcs[2], g3qps[2],
                   p2[4], g3kplcs[3], g3qps[3],
                   p2[5], g3kplcs[4], g3qps[4],
                   p2[6], g3kplcs[5], g3qps[5],
                   p2[7], g3kplcs[6], g3qps[6],
                   g3kplcs[7], g3tp, g3qps[7]]
            scan(2, units=sc2, upc=3)

            # sc3: preps(3) + wo folded in
            p3 = preps(3)
            p3[0]()
            # wo(lt) reads att chunks 2lt,2lt+1 -> MUST be emitted after
            # nd(3, 2lt+1), i.e. in chunk-(2lt+1) slots
            sc3 = [p3[1], p3[2],
                   wo_unit(0, (0,)), p3[3],
                   wo_unit(0, (1,)), p3[4],
                   wo_unit(1, (0,)), p3[5],
                   wo_unit(1, (1,)), p3[6],
                   wo_unit(2, (0,)), p3[7],
                   wo_unit(2, (1,)), nop,
                   wo_unit(3, (0,)), wo_unit(3, (1,))]
            scan(3, units=sc3, upc=2)
            if _DEBUG:
                nc.sync.dma_start(dbg["att0"], t_att[0][:, :])
                nc.sync.dma_start(dbg["att1"], t_att[1][:, :])
                nc.sync.dma_start(dbg["vas"], t_vs[:, :])
                nc.sync.dma_start(dbg["S0"], heads[0][3][:, :])
            psSb_c.__exit__(None, None, None)
            psG2_c.__exit__(None, None, None)
            psND_c.__exit__(None, None, None)
            psA_c.__exit__(None, None, None)
            octx.__exit__(None, None, None)
            for c in reversed(hctx):
                c.__exit__(None, None, None)

    nc.compile()
    return nc


def _prep_inputs(query, key, value, Wq, bq, Wk, bk, Wv, bv, Wo, bo, proj):
    from ml_dtypes import bfloat16
    s = float(DK) ** -0.25

    def bf(x):
        return np.ascontiguousarray(x).astype(bfloat16)

    tri = (np.arange(128)[:, None] <= np.arange(128)[None, :]).astype(
        np.float32)
    on = np.ones((128, 128), np.float32)
    msk = np.concatenate([tri, on, tri], axis=1)
    bd = np.zeros((128, 8), np.float32)
    for half in range(2):
        for r in range(128):
            bd[r, 4 * half + (2 * half + r // 64)] = -0.5
    pT = np.ascontiguousarray(proj.T)
    projT2 = np.concatenate([pT, pT])
    in_maps = []
    for b in range(B):
        for hg in range(2):
            sl = slice(hg * 256, (hg + 1) * 256)

            def hpack(mat):
                k = mat.shape[0] // 128
                return np.concatenate([mat[128 * i:128 * (i + 1)]
                                       for i in range(k)], axis=1)

            blob = np.zeros((128, _CB), np.float32)
            blob[:, _OFF_WQ:_OFF_WQ + 1024] = hpack(Wq[sl].T * s)
            blob[:, _OFF_WK:_OFF_WK + 1024] = hpack(Wk[sl].T * s)
            blob[:, _OFF_BQK + 0] = bq[sl][:128] * s
            blob[:, _OFF_BQK + 1] = bq[sl][128:] * s
            blob[:, _OFF_BQK + 2] = bk[sl][:128] * s
            blob[:, _OFF_BQK + 3] = bk[sl][128:] * s
            blob[:, _OFF_PROJT:_OFF_PROJT + 256] = projT2
            blob[:, _OFF_BD:_OFF_BD + 8] = bd
            blob[:, _OFF_WV:_OFF_WV + 1024] = hpack(Wv[sl].T)
            blob[:, _OFF_MSK:_OFF_MSK + 384] = msk
            blob[:, _OFF_WO2:_OFF_WO2 + 1024] = hpack(Wo[:, sl].T)
            m = {"cblob": bf(blob),
                 "xq": bf(query[b].T),
                 "xk": bf(key[b].T),
                 "xv": bf(value[b].T)}
            in_maps.append(m)
    return in_maps


def kernel(query, key, value, Wq, bq, Wk, bk, Wv, bv, Wo, bo, proj,
           _trace=False):
    global _COMPILED
    from concourse import bass_utils
    args = [np.asarray(a, np.float32) for a in
            (query, key, value, Wq, bq, Wk, bk, Wv, bv, Wo, bo, proj)]
    if _COMPILED is None:
        _COMPILED = _build()
    in_maps = _prep_inputs(*args)
    res = bass_utils.run_bass_kernel_spmd(
        _COMPILED, in_maps, core_ids=list(range(8)), trace=_trace)
    out = np.empty((B, L, DIM), np.float32)
    Wo_, bv_, bo_ = args[9], args[8], args[10]
    cvec = Wo_ @ bv_ + bo_
    for b in range(B):
        out[b] = (res.results[2 * b]["outT"].astype(np.float32).T
                  + res.results[2 * b + 1]["outT"].astype(np.float32).T
                  + cvec)
    if _trace:
        kernel._last = res
    return out
